# revision 24
# baseline (speedup 1.0000x reference)
"""8-core Trainium2 Bass kernel for the ARMA GNN problem (nn_ARMA_15453292331025).

Architecture (per NeuronCore, SPMD over 8 cores):
  - Nodes are partitioned into 8 blocks of 6250 (core c owns destinations
    [6250c, 6250(c+1))). Each core processes exactly the edges whose
    destination lies in its block.
  - Message passing (gather + weighted segment-sum) is done as:
      * dma_gather of source rows from a DRAM table (bf16, 128 rows/tile)
      * per 128-edge tile, a one-hot(dst)*norm matrix is built on DVE and a
        TensorE matmul contracts edges -> per-destination partial sums,
        accumulated in PSUM per 128-destination chunk.
  - The linear weights commute with aggregation for conv1, so t=0 gathers x
    itself and t=1 gathers out0; the [128,128] weights are applied to the
    128-row aggregate via a TensorE transpose + matmul.
  - Cross-core exchange is 3 AllGathers of bf16 node tables (out0 [N,384],
    h2_0 [N,128-padded], h2_1 [N,128-padded]).
  - Graph pooling (segment-sum over sorted batch ids) is a one-hot matmul
    accumulated over chunks; per-core partials are summed on the host.

kernel(**inputs) takes the full (unsharded) inputs and returns the full
(out [64,8], features [64,128]) tuple, matching the reference.
"""

import os
import sys

import numpy as np

sys.path.insert(0, "/opt/trn_rl_repo")

import ml_dtypes

bf16 = ml_dtypes.bfloat16

# ---------------------------------------------------------------- constants
N = 50000
F_IN = 128
HID = 128
CLS = 8
K = 3
G = 64
NCORES = 8
CHUNK = 128
GROUP = 2          # dst chunks per gather group
W1X = K * HID      # 384
W2X = K * CLS      # 24
HALF_SPLIT = 2     # int16 index range split

DEF_CFG = dict(N=N, F=F_IN, HID=HID, CLS=CLS, K=K, G=G,
               NCORES=NCORES, CHUNK=CHUNK, GROUP=GROUP)


# ---------------------------------------------------------------- host prep
def host_prep(edge_index, cfg):
    """Build the uniform (cross-core identical) tile structure and per-core
    gather/metadata streams."""
    n = cfg["N"]
    ncores = cfg["NCORES"]
    npc = n // ncores
    nchunk = (npc + cfg["CHUNK"] - 1) // cfg["CHUNK"]
    half = (n // 2 + 15) // 16 * 16  # split point for int16 indices

    row = np.asarray(edge_index[0], np.int64)
    col = np.asarray(edge_index[1], np.int64)
    deg = np.bincount(col, minlength=n).astype(np.float32)
    dis = np.where(deg > 0, 1.0 / np.sqrt(np.maximum(deg, 1.0)), 0.0).astype(np.float32)
    norm = (dis[row] * dis[col]).astype(np.float32)

    per_core = []
    counts = np.zeros((ncores, nchunk, 2), np.int64)
    for c in range(ncores):
        lo = c * npc
        m = (col >= lo) & (col < lo + npc)
        er, ec, en = row[m], col[m] - lo, norm[m]
        ch = ec // cfg["CHUNK"]
        hf = (er >= half).astype(np.int64)
        key = ch * 2 + hf
        order = np.argsort(key, kind="stable")
        er, ec, en, key = er[order], ec[order], en[order], key[order]
        cnt = np.bincount(key, minlength=nchunk * 2).reshape(nchunk, 2)
        counts[c] = cnt
        per_core.append((er, ec, en, cnt))

    tiles = np.maximum((counts.max(axis=0) + 127) // 128, 0)
    # ensure at least one tile per chunk (half 0) so PSUM gets initialized
    zero_rows = tiles.sum(axis=1) == 0
    tiles[zero_rows, 0] = 1

    T = [int(tiles[:, h].sum()) for h in range(2)]           # tiles per half
    S = [t * 128 for t in T]                                  # slots per half

    idx_np = [np.zeros((ncores, 128, S[h] // 16), np.int16) for h in range(2)]
    nrm_np = [np.zeros((ncores, 128, T[h]), bf16) for h in range(2)]
    dst_np = [np.zeros((ncores, 128, T[h]), bf16) for h in range(2)]

    for c in range(ncores):
        er, ec, en, cnt = per_core[c]
        # slot offsets per (chunk, half) in the uniform layout
        epos = np.concatenate([[0], np.cumsum(cnt.reshape(-1))])  # within core stream
        for h in range(2):
            ivals = np.zeros(S[h], np.int64)
            nvals = np.zeros(S[h], np.float32)
            dvals = np.zeros(S[h], np.float32)
            t_off = 0
            for ch in range(nchunk):
                k = ch * 2 + h
                cn = cnt[ch, h]
                s0 = t_off * 128
                e0 = epos[k]
                ivals[s0:s0 + cn] = er[e0:e0 + cn] - h * half
                nvals[s0:s0 + cn] = en[e0:e0 + cn]
                dvals[s0:s0 + cn] = ec[e0:e0 + cn] - ch * cfg["CHUNK"]
                t_off += tiles[ch, h]
            wrapped = ivals.astype(np.int16).reshape(-1, 16).T
            idx_np[h][c] = np.tile(wrapped, (8, 1))  # replicate per 16-part group
            nrm_np[h][c] = nvals.astype(bf16).reshape(-1, 128).T
            dst_np[h][c] = dvals.astype(bf16).reshape(-1, 128).T

    # groups of chunks
    groups = []
    g0 = 0
    while g0 < nchunk:
        g1 = min(g0 + cfg["GROUP"], nchunk)
        groups.append((g0, g1))
        g0 = g1
    return dict(tiles=tiles, groups=groups, idx=idx_np, nrm=nrm_np, dst=dst_np,
                T=T, npc=npc, nchunk=nchunk, half=half)


def pack_weights(inputs, cfg):
    """Host-side packing of the small ARMA weights into lhsT/rhs layouts."""
    k, hid, cls, f = cfg["K"], cfg["HID"], cfg["CLS"], cfg["F"]
    w1x, w2x = k * hid, k * cls

    def t(a):
        return np.asarray(a, np.float32)

    W0 = np.transpose(t(inputs["c1_init_w"]), (1, 0, 2)).reshape(f, w1x)
    R1 = np.transpose(t(inputs["c1_root_w"]), (1, 0, 2)).reshape(f, w1x)
    W1 = np.transpose(t(inputs["c1_w"]), (1, 0, 2)).reshape(hid, w1x)
    B1 = np.broadcast_to(t(inputs["c1_bias"]).reshape(1, w1x), (128, w1x))
    W20 = np.zeros((hid, 128), np.float32)
    W20[:, :w2x] = np.transpose(t(inputs["c2_init_w"]), (1, 0, 2)).reshape(hid, w2x)
    R2 = np.zeros((hid, 128), np.float32)
    R2[:, :w2x] = np.transpose(t(inputs["c2_root_w"]), (1, 0, 2)).reshape(hid, w2x)
    W2 = np.zeros((32, 128), np.float32)
    for kk in range(k):
        W2[kk * cls:(kk + 1) * cls, kk * cls:(kk + 1) * cls] = t(inputs["c2_w"])[kk]
    B2 = np.zeros((128, 128), np.float32)
    B2[:, :w2x] = np.broadcast_to(t(inputs["c2_bias"]).reshape(1, w2x), (128, w2x))
    return {n: v.astype(bf16) for n, v in
            dict(W0=W0, R1=R1, W1=W1, B1=B1, W20=W20, R2=R2, W2=W2, B2=B2).items()}


# ---------------------------------------------------------------- builder
def build_kernel(cfg, st, debug=False):
    import concourse.bass as bass
    import concourse.mybir as mybir
    import concourse.tile as tile
    from concourse import bacc, library_config

    n, f, hid, cls, k, g = cfg["N"], cfg["F"], cfg["HID"], cfg["CLS"], cfg["K"], cfg["G"]
    ncores = cfg["NCORES"]
    npc, nchunk = st["npc"], st["nchunk"]
    half = st["half"]
    w1x, w2x = k * hid, k * cls
    tiles, groups = st["tiles"], st["groups"]
    T = st["T"]
    dt = mybir.dt
    AOT = mybir.AluOpType
    ACT = mybir.ActivationFunctionType

    nc = bacc.Bacc(None, target_bir_lowering=False, num_devices=ncores)

    # ---------------- dram parameters
    def din(name, shape, dtype):
        return nc.dram_tensor(name, shape, dtype, kind="ExternalInput")

    xtab = din("xtab", [n, f], dt.bfloat16)
    xT = din("xT", [f, npc], dt.bfloat16)
    iota128p = din("iota128", [128, 128], dt.bfloat16)
    iota64p = din("iota64", [128, g], dt.bfloat16)
    identp = din("ident", [128, 128], dt.bfloat16)
    wts = {nm: din(nm, list(sh), dt.bfloat16) for nm, sh in dict(
        W0=(f, w1x), R1=(f, w1x), W1=(hid, w1x), B1=(128, w1x),
        W20=(hid, 128), R2=(hid, 128), W2=(32, 128), B2=(128, 128)).items()}
    idxp = [din(f"idx{h}", [128, T[h] * 8], dt.int16) for h in range(2)]
    nrmp = [din(f"nrm{h}", [128, T[h]], dt.bfloat16) for h in range(2)]
    dstp = [din(f"dst{h}", [128, T[h]], dt.bfloat16) for h in range(2)]
    gidp = din("gid", [128, nchunk], dt.bfloat16)

    out_part = nc.dram_tensor("out_part", [g, cls], dt.float32, kind="ExternalOutput")
    feat_part = nc.dram_tensor("feat_part", [g, hid], dt.float32, kind="ExternalOutput")
    if debug:
        dbg = {nm: nc.dram_tensor(nm, sh, dt.bfloat16, kind="ExternalOutput")
               for nm, sh in dict(d_out0=[n, w1x], d_h20=[n, 128], d_h21=[n, 128],
                                  d_hblk=[128, nchunk * hid],
                                  d_root1=[128, nchunk * w1x],
                                  d_root2=[128, nchunk * 128]).items()}

    rg = [list(range(ncores))]

    with tile.TileContext(nc) as tc:
        import contextlib
        with contextlib.ExitStack() as ctx:
            nc.gpsimd.load_library(library_config.mlp)
            dram = ctx.enter_context(tc.tile_pool(name="dram", bufs=1, space="DRAM"))
            cc1_in = dram.tile([npc, w1x], dt.bfloat16)
            cc1_out = dram.tile([n, w1x], dt.bfloat16, addr_space="Shared")
            cc2_in = dram.tile([npc, 128], dt.bfloat16)
            cc2_out = dram.tile([n, 128], dt.bfloat16, addr_space="Shared")
            cc3_in = dram.tile([npc, 128], dt.bfloat16)
            cc3_out = dram.tile([n, 128], dt.bfloat16, addr_space="Shared")

            const = ctx.enter_context(tc.tile_pool(name="const", bufs=1))
            iota128 = const.tile([128, 128], dt.bfloat16)
            nc.sync.dma_start(out=iota128[:], in_=iota128p[:])
            iota64 = const.tile([128, g], dt.bfloat16)
            nc.sync.dma_start(out=iota64[:], in_=iota64p[:])
            ident = const.tile([128, 128], dt.bfloat16)
            nc.sync.dma_start(out=ident[:], in_=identp[:])

            wt_sb = {}
            for nm, t_ in wts.items():
                s = const.tile(list(t_.shape), dt.bfloat16, name=f"sb_{nm}")
                nc.sync.dma_start(out=s[:], in_=t_[:])
                wt_sb[nm] = s
            xT_sb = const.tile([f, npc], dt.bfloat16)
            nc.sync.dma_start(out=xT_sb[:], in_=xT[:])
            idx_sb = []
            nrm_sb = []
            dst_sb = []
            for h in range(2):
                i_ = const.tile([128, T[h] * 8], dt.int16, name=f"idx_sb{h}")
                nc.sync.dma_start(out=i_[:], in_=idxp[h][:])
                nm_ = const.tile([128, T[h]], dt.bfloat16, name=f"nrm_sb{h}")
                nc.sync.dma_start(out=nm_[:], in_=nrmp[h][:])
                ds_ = const.tile([128, T[h]], dt.bfloat16, name=f"dst_sb{h}")
                nc.sync.dma_start(out=ds_[:], in_=dstp[h][:])
                idx_sb.append(i_)
                nrm_sb.append(nm_)
                dst_sb.append(ds_)
            gid_sb = const.tile([128, nchunk], dt.bfloat16)
            nc.sync.dma_start(out=gid_sb[:], in_=gidp[:])

            big = ctx.enter_context(tc.tile_pool(name="big", bufs=1))
            root1_sb = big.tile([128, nchunk * w1x], dt.bfloat16)
            root2_sb = big.tile([128, nchunk * 128], dt.bfloat16)
            hblk_sb = big.tile([128, nchunk * hid], dt.bfloat16)

            sbuf = ctx.enter_context(tc.tile_pool(name="sbuf", bufs=3))
            lhsp = ctx.enter_context(tc.tile_pool(name="lhsp", bufs=4))
            gbp = ctx.enter_context(tc.tile_pool(name="gbp", bufs=2))
            p_agg = ctx.enter_context(tc.tile_pool(name="p_agg", bufs=2, space="PSUM"))
            p_z = ctx.enter_context(tc.tile_pool(name="p_z", bufs=2, space="PSUM"))
            p_tp = ctx.enter_context(tc.tile_pool(name="p_tp", bufs=2, space="PSUM"))
            p_acc = ctx.enter_context(tc.tile_pool(name="p_acc", bufs=1, space="PSUM"))

            def cw_of(ch):
                return min(cfg["CHUNK"], npc - ch * cfg["CHUNK"])

            # ---------------- root1 = xT.T @ R1 + B1  (bf16, per chunk)
            for ch in range(nchunk):
                cw = cw_of(ch)
                rp = p_z.tile([128, w1x], dt.float32, tag="z")
                nc.tensor.matmul(out=rp[:cw, :], lhsT=xT_sb[:, ch * 128: ch * 128 + cw],
                                 rhs=wt_sb["R1"][:], start=True, stop=True)
                nc.vector.tensor_tensor(out=rp[:cw, :], in0=rp[:cw, :],
                                        in1=wt_sb["B1"][:cw, :], op=AOT.add)
                if cw < 128:
                    nc.vector.memset(root1_sb[:, ch * w1x:(ch + 1) * w1x], 0.0)
                nc.scalar.activation(out=root1_sb[:cw, ch * w1x:(ch + 1) * w1x],
                                     in_=rp[:cw, :], func=ACT.Copy)

            # ---------------- generic propagate phase
            def propagate(phase, table, elem, msg_w, epilogue):
                """table: dram AP [n, elem]; per chunk calls
                epilogue(ch, cw, agg_psum_ap [128, msg_w] fp32)."""
                # tile offset within each half stream, per chunk
                t_off = np.zeros((nchunk, 2), np.int64)
                t_off[1:] = np.cumsum(tiles[:-1], axis=0)
                MAXT = 8  # max tiles (1024 idxs) per dma_gather call (SWDGE ring cap)
                for (g0, g1) in groups:
                    gt = [int(tiles[g0:g1, h].sum()) for h in range(2)]
                    gb = []
                    for h in range(2):
                        nt = max(gt[h], 1)
                        b = gbp.tile([128, nt, elem], dt.bfloat16,
                                     tag=f"gb{elem}_{h}", name=f"gb{phase}_{h}_{g0}")
                        base = 0 if h == 0 else half
                        rows = (n - half) if h == 1 else half
                        for c0 in range(0, gt[h], MAXT):
                            c1 = min(c0 + MAXT, gt[h])
                            s0 = (int(t_off[g0, h]) + c0) * 128
                            nidx = (c1 - c0) * 128
                            nc.gpsimd.dma_gather(
                                out_ap=b[:, c0:c1, :],
                                in_ap=table[base:base + rows, :],
                                idxs_ap=idx_sb[h][:, s0 // 16:(s0 + nidx) // 16],
                                num_idxs=nidx,
                                num_idxs_reg=nidx,
                                elem_size=elem,
                            )
                        gb.append(b)
                    for ch in range(g0, g1):
                        cw = cw_of(ch)
                        agg = p_agg.tile([128, w1x], dt.float32, tag="agg",
                                         name=f"agg{phase}_{ch}")
                        ntot = int(tiles[ch].sum())
                        ti = 0
                        for h in range(2):
                            for t_ in range(int(tiles[ch, h])):
                                gtile = int(t_off[ch, h] - t_off[g0, h] + t_)
                                stile = int(t_off[ch, h] + t_)
                                lhsT = lhsp.tile([128, 128], dt.bfloat16,
                                                 tag="lhsT", name=f"lh{phase}_{ch}_{h}_{t_}")
                                nc.vector.scalar_tensor_tensor(
                                    out=lhsT[:],
                                    in0=iota128[:],
                                    scalar=dst_sb[h][:, stile:stile + 1],
                                    in1=nrm_sb[h][:, stile:stile + 1].to_broadcast([128, 128]),
                                    op0=AOT.is_equal, op1=AOT.mult)
                                nc.tensor.matmul(
                                    out=agg[:, :msg_w], lhsT=lhsT[:],
                                    rhs=gb[h][:, gtile, :msg_w],
                                    start=(ti == 0), stop=(ti == ntot - 1))
                                ti += 1
                        epilogue(ch, cw, agg)

            # ---------------- conv1 t=0: gather x, agg, @W0, +root1, relu
            def epi_c1t0(ch, cw, agg):
                aggS = sbuf.tile([128, f], dt.bfloat16, tag="aggS", name=f"aggS0_{ch}")
                nc.scalar.activation(out=aggS[:], in_=agg[:, :f], func=ACT.Copy)
                aT = p_tp.tile([128, 128], dt.bfloat16, tag="tp", name=f"aT0_{ch}")
                nc.tensor.transpose(out=aT[:], in_=aggS[:], identity=ident[:])
                aTs = sbuf.tile([128, 128], dt.bfloat16, tag="aTs", name=f"aTs0_{ch}")
                nc.vector.tensor_copy(aTs[:], aT[:])
                z = p_z.tile([128, w1x], dt.float32, tag="z", name=f"z0_{ch}")
                nc.tensor.matmul(out=z[:], lhsT=aTs[:], rhs=wt_sb["W0"][:],
                                 start=True, stop=True)
                nc.vector.tensor_tensor(out=z[:cw, :], in0=z[:cw, :],
                                        in1=root1_sb[:cw, ch * w1x:(ch + 1) * w1x],
                                        op=AOT.add)
                o0 = sbuf.tile([128, w1x], dt.bfloat16, tag="obuf", name=f"o0_{ch}")
                nc.scalar.activation(out=o0[:cw, :], in_=z[:cw, :], func=ACT.Relu)
                nc.sync.dma_start(out=cc1_in[ch * 128: ch * 128 + cw, :], in_=o0[:cw, :])

            propagate("c1t0", xtab, f, f, epi_c1t0)

            nc.gpsimd.collective_compute(
                "AllGather", AOT.bypass, replica_groups=rg,
                ins=[cc1_in.opt()], outs=[cc1_out.opt()])

            # ---------------- conv1 t=1
            def epi_c1t1(ch, cw, agg):
                aggS = sbuf.tile([128, w1x], dt.bfloat16, tag="aggS", name=f"aggS1_{ch}")
                nc.scalar.activation(out=aggS[:], in_=agg[:, :w1x], func=ACT.Copy)
                z = p_z.tile([128, w1x], dt.float32, tag="z", name=f"z1_{ch}")
                for kk in range(k):
                    sl = slice(kk * hid, (kk + 1) * hid)
                    aT = p_tp.tile([128, 128], dt.bfloat16, tag="tp", name=f"aT1_{ch}_{kk}")
                    nc.tensor.transpose(out=aT[:], in_=aggS[:, sl], identity=ident[:])
                    aTs = sbuf.tile([128, 128], dt.bfloat16, tag="aTs", name=f"aTs1_{ch}_{kk}")
                    nc.vector.tensor_copy(aTs[:], aT[:])
                    nc.tensor.matmul(out=z[:, sl], lhsT=aTs[:], rhs=wt_sb["W1"][:, sl],
                                     start=True, stop=True)
                nc.vector.tensor_tensor(out=z[:cw, :], in0=z[:cw, :],
                                        in1=root1_sb[:cw, ch * w1x:(ch + 1) * w1x],
                                        op=AOT.add)
                o1 = sbuf.tile([128, w1x], dt.bfloat16, tag="obuf", name=f"o1_{ch}")
                nc.scalar.activation(out=o1[:], in_=z[:], func=ACT.Relu)
                # h = mean over stacks (padded rows of z are zero+root(=0) -> relu 0)
                tmp = sbuf.tile([128, hid], dt.float32, tag="tmp", name=f"tm1_{ch}")
                nc.vector.tensor_tensor(out=tmp[:], in0=o1[:, 0:hid],
                                        in1=o1[:, hid:2 * hid], op=AOT.add)
                nc.vector.tensor_tensor(out=tmp[:], in0=tmp[:],
                                        in1=o1[:, 2 * hid:3 * hid], op=AOT.add)
                hsl = hblk_sb[:, ch * hid:(ch + 1) * hid]
                nc.vector.tensor_scalar_mul(hsl, tmp[:], 1.0 / 3.0)
                # conv2 prework on this chunk
                hT = p_tp.tile([128, 128], dt.bfloat16, tag="tp", name=f"hT_{ch}")
                nc.tensor.transpose(out=hT[:], in_=hsl, identity=ident[:])
                hTs = sbuf.tile([128, 128], dt.bfloat16, tag="aTs", name=f"hTs_{ch}")
                nc.vector.tensor_copy(hTs[:], hT[:])
                h20 = p_z.tile([128, w1x], dt.float32, tag="z", name=f"h20_{ch}")
                nc.tensor.matmul(out=h20[:, :128], lhsT=hTs[:], rhs=wt_sb["W20"][:],
                                 start=True, stop=True)
                o20 = sbuf.tile([128, 128], dt.bfloat16, tag="o2buf", name=f"o20_{ch}")
                nc.scalar.activation(out=o20[:], in_=h20[:, :128], func=ACT.Copy)
                nc.sync.dma_start(out=cc2_in[ch * 128: ch * 128 + cw, :], in_=o20[:cw, :])
                r2 = p_z.tile([128, w1x], dt.float32, tag="z", name=f"r2_{ch}")
                nc.tensor.matmul(out=r2[:cw, :128], lhsT=hTs[:, :cw],
                                 rhs=wt_sb["R2"][:], start=True, stop=True)
                nc.vector.tensor_tensor(out=r2[:cw, :128], in0=r2[:cw, :128],
                                        in1=wt_sb["B2"][:cw, :], op=AOT.add)
                if cw < 128:
                    nc.vector.memset(root2_sb[:, ch * 128:(ch + 1) * 128], 0.0)
                nc.scalar.activation(out=root2_sb[:cw, ch * 128:(ch + 1) * 128],
                                     in_=r2[:cw, :128], func=ACT.Copy)
                # features pool
                oh = lhsp.tile([128, g], dt.bfloat16, tag="oh", name=f"oh_{ch}")
                nc.vector.tensor_tensor(out=oh[:], in0=gid_sb[:, ch:ch + 1].to_broadcast([128, g]),
                                        in1=iota64[:], op=AOT.is_equal)
                nc.tensor.matmul(out=featp[:, :], lhsT=oh[:], rhs=hsl,
                                 start=(ch == 0), stop=(ch == nchunk - 1),
                                 skip_group_check=True)

            featp = p_acc.tile([g, hid], dt.float32, tag="featp")
            propagate("c1t1", cc1_out, w1x, w1x, epi_c1t1)

            nc.gpsimd.collective_compute(
                "AllGather", AOT.bypass, replica_groups=rg,
                ins=[cc2_in.opt()], outs=[cc2_out.opt()])

            # ---------------- conv2 t=0: gather h2_0 table, z=agg+root2 -> h2_1
            def epi_c2t0(ch, cw, agg):
                z = sbuf.tile([128, w2x], dt.bfloat16, tag="tmp2", name=f"z20_{ch}")
                nc.vector.tensor_tensor(out=z[:], in0=agg[:, :w2x],
                                        in1=root2_sb[:, ch * 128: ch * 128 + w2x],
                                        op=AOT.add)
                oT = p_tp.tile([128, 128], dt.bfloat16, tag="tp", name=f"oT2_{ch}")
                nc.tensor.transpose(out=oT[:w2x, :], in_=z[:], identity=ident[:])
                oTs = sbuf.tile([32, 128], dt.bfloat16, tag="oTs", name=f"oTs2_{ch}")
                nc.vector.tensor_copy(oTs[:w2x, :], oT[:w2x, :])
                h21 = p_z.tile([128, w1x], dt.float32, tag="z", name=f"h21_{ch}")
                nc.tensor.matmul(out=h21[:, :128], lhsT=oTs[:w2x, :],
                                 rhs=wt_sb["W2"][:w2x, :], start=True, stop=True)
                o21 = sbuf.tile([128, 128], dt.bfloat16, tag="o2buf", name=f"o21_{ch}")
                nc.scalar.activation(out=o21[:], in_=h21[:, :128], func=ACT.Copy)
                nc.sync.dma_start(out=cc3_in[ch * 128: ch * 128 + cw, :], in_=o21[:cw, :])

            propagate("c2t0", cc2_out, 128, w2x, epi_c2t0)

            nc.gpsimd.collective_compute(
                "AllGather", AOT.bypass, replica_groups=rg,
                ins=[cc3_in.opt()], outs=[cc3_out.opt()])

            # ---------------- conv2 t=1: z=agg+root2, mean stacks, pool
            def epi_c2t1(ch, cw, agg):
                z2 = sbuf.tile([128, w2x], dt.float32, tag="z2t1", name=f"z2t1_{ch}")
                nc.vector.tensor_tensor(out=z2[:], in0=agg[:, :w2x],
                                        in1=root2_sb[:, ch * 128: ch * 128 + w2x],
                                        op=AOT.add)
                tmp = sbuf.tile([128, cls], dt.float32, tag="tmp2", name=f"tm2_{ch}")
                nc.vector.tensor_tensor(out=tmp[:], in0=z2[:, 0:cls],
                                        in1=z2[:, cls:2 * cls], op=AOT.add)
                nc.vector.tensor_tensor(out=tmp[:], in0=tmp[:],
                                        in1=z2[:, 2 * cls:3 * cls], op=AOT.add)
                zm = sbuf.tile([128, cls], dt.bfloat16, tag="zm", name=f"zm_{ch}")
                nc.vector.tensor_scalar_mul(zm[:], tmp[:], 1.0 / 3.0)
                oh = lhsp.tile([128, g], dt.bfloat16, tag="oh", name=f"oh2_{ch}")
                nc.vector.tensor_tensor(out=oh[:], in0=gid_sb[:, ch:ch + 1].to_broadcast([128, g]),
                                        in1=iota64[:], op=AOT.is_equal)
                nc.tensor.matmul(out=outp[:, :], lhsT=oh[:], rhs=zm[:],
                                 start=(ch == 0), stop=(ch == nchunk - 1),
                                 skip_group_check=True)

            outp = p_acc.tile([g, cls], dt.float32, tag="outp")
            propagate("c2t1", cc3_out, 128, w2x, epi_c2t1)

            if debug:
                nc.sync.dma_start(out=dbg["d_out0"][:], in_=cc1_out[:])
                nc.sync.dma_start(out=dbg["d_h20"][:], in_=cc2_out[:])
                nc.sync.dma_start(out=dbg["d_h21"][:], in_=cc3_out[:])
                nc.sync.dma_start(out=dbg["d_hblk"][:], in_=hblk_sb[:])
                nc.sync.dma_start(out=dbg["d_root1"][:], in_=root1_sb[:])
                nc.sync.dma_start(out=dbg["d_root2"][:], in_=root2_sb[:])

            # ---------------- finals
            featS = sbuf.tile([g, hid], dt.float32, tag="featS")
            nc.vector.tensor_copy(featS[:], featp[:])
            nc.sync.dma_start(out=feat_part[:], in_=featS[:])
            outS = sbuf.tile([g, cls], dt.float32, tag="outS")
            nc.vector.tensor_copy(outS[:], outp[:])
            nc.sync.dma_start(out=out_part[:], in_=outS[:])

    nc.compile()
    return nc


# ---------------------------------------------------------------- in_maps
def build_in_maps(inputs, cfg, st):
    n, f = cfg["N"], cfg["F"]
    ncores = cfg["NCORES"]
    npc = st["npc"]
    nchunk = st["nchunk"]
    x = np.asarray(inputs["x"], np.float32)
    batch = np.asarray(inputs["batch"], np.int64)
    wt = pack_weights(inputs, cfg)
    x_bf = x.astype(bf16)

    in_maps = []
    ar = np.arange(128, dtype=np.float32)
    iota128 = np.broadcast_to(ar[None, :], (128, 128)).astype(bf16)
    iota64 = np.broadcast_to(ar[None, :cfg["G"]], (128, cfg["G"])).astype(bf16)
    ident = np.eye(128, dtype=np.float32).astype(bf16)
    for c in range(ncores):
        lo = c * npc
        m = dict(xtab=x_bf,
                 xT=np.ascontiguousarray(x_bf[lo:lo + npc].T),
                 iota128=iota128, iota64=iota64, ident=ident)
        m.update(wt)
        for h in range(2):
            m[f"idx{h}"] = np.ascontiguousarray(st["idx"][h][c])
            m[f"nrm{h}"] = np.ascontiguousarray(st["nrm"][h][c])
            m[f"dst{h}"] = np.ascontiguousarray(st["dst"][h][c])
        gid = np.zeros((128, nchunk), bf16)
        gv = batch[lo:lo + npc].astype(np.float32)
        gv = np.concatenate([gv, np.zeros(nchunk * 128 - npc, np.float32)])
        gid[:, :] = gv.reshape(nchunk, 128).T.astype(bf16)
        m["gid"] = gid
        in_maps.append(m)
    return in_maps


_CACHE = {}


def kernel(**inputs):
    from concourse.bass_utils import run_bass_kernel_spmd

    cfg = DEF_CFG
    edge_index = np.asarray(inputs["edge_index"])
    st = host_prep(edge_index, cfg)
    nc = build_kernel(cfg, st)
    in_maps = build_in_maps(inputs, cfg, st)
    res = run_bass_kernel_spmd(nc, in_maps, core_ids=list(range(cfg["NCORES"])))
    out = np.zeros((cfg["G"], cfg["CLS"]), np.float32)
    feat = np.zeros((cfg["G"], cfg["HID"]), np.float32)
    for r in res.results:
        out += np.asarray(r["out_part"], np.float32)
        feat += np.asarray(r["feat_part"], np.float32)
    return out, feat


# revision 35
# speedup vs baseline: 1.0868x; 1.0868x over previous
"""8-core Trainium2 Bass kernel for the ARMA GNN problem (nn_ARMA_15453292331025).

Architecture (per NeuronCore, SPMD over 8 cores):
  - Nodes are partitioned into 8 blocks of 6250 (core c owns destinations
    [6250c, 6250(c+1))). Each core processes exactly the edges whose
    destination lies in its block.
  - Message passing (gather + weighted segment-sum) is done as:
      * dma_gather of source rows from a DRAM table (bf16, 128 rows/tile)
      * per 128-edge tile, a one-hot(dst)*norm matrix is built on DVE and a
        TensorE matmul contracts edges -> per-destination partial sums,
        accumulated in PSUM per 128-destination chunk.
  - The linear weights commute with aggregation for conv1, so t=0 gathers x
    itself and t=1 gathers out0; the [128,128] weights are applied to the
    128-row aggregate via a TensorE transpose + matmul.
  - Cross-core exchange is 3 AllGathers of bf16 node tables (out0 [N,384],
    h2_0 [N,128-padded], h2_1 [N,128-padded]).
  - Graph pooling (segment-sum over sorted batch ids) is a one-hot matmul
    accumulated over chunks; per-core partials are summed on the host.

kernel(**inputs) takes the full (unsharded) inputs and returns the full
(out [64,8], features [64,128]) tuple, matching the reference.
"""

import os
import sys

import numpy as np

sys.path.insert(0, "/opt/trn_rl_repo")

import ml_dtypes

bf16 = ml_dtypes.bfloat16

# ---------------------------------------------------------------- constants
N = 50000
F_IN = 128
HID = 128
CLS = 8
K = 3
G = 64
NCORES = 8
CHUNK = 128
GROUP = 2          # dst chunks per gather group
W1X = K * HID      # 384
W2X = K * CLS      # 24
HALF_SPLIT = 2     # int16 index range split

DEF_CFG = dict(N=N, F=F_IN, HID=HID, CLS=CLS, K=K, G=G,
               NCORES=NCORES, CHUNK=CHUNK, GROUP=GROUP)

# Pad gather slots with idx=-1 (ucode trims trailing negatives, skipping the
# descriptor work). The interp models -1 differently, so sim tests set False.
PAD_NEG = True
MAXT = 8  # max tiles (1024 idxs) per dma_gather call (SWDGE ring capacity)


def enumerate_calls(st, groups):
    """Device-order gather call list: (h, ch, tile0, tile1) per group."""
    tiles = st["tiles"]
    calls = []
    for (g0, g1) in groups:
        for h in range(2):
            for ch in range(g0, g1):
                ct = int(tiles[ch, h])
                for c0 in range(0, ct, MAXT):
                    calls.append((h, ch, c0, min(c0 + MAXT, ct)))
    return calls


# ---------------------------------------------------------------- host prep
def host_prep(edge_index, cfg):
    """Build the uniform (cross-core identical) tile structure and per-core
    gather/metadata streams."""
    n = cfg["N"]
    ncores = cfg["NCORES"]
    npc = n // ncores
    nchunk = (npc + cfg["CHUNK"] - 1) // cfg["CHUNK"]
    half = (n // 2 + 15) // 16 * 16  # split point for int16 indices

    row = np.asarray(edge_index[0], np.int64)
    col = np.asarray(edge_index[1], np.int64)
    deg = np.bincount(col, minlength=n).astype(np.float32)
    dis = np.where(deg > 0, 1.0 / np.sqrt(np.maximum(deg, 1.0)), 0.0).astype(np.float32)
    norm = (dis[row] * dis[col]).astype(np.float32)

    per_core = []
    counts = np.zeros((ncores, nchunk, 2), np.int64)
    for c in range(ncores):
        lo = c * npc
        m = (col >= lo) & (col < lo + npc)
        er, ec, en = row[m], col[m] - lo, norm[m]
        ch = ec // cfg["CHUNK"]
        hf = (er >= half).astype(np.int64)
        key = ch * 2 + hf
        order = np.argsort(key, kind="stable")
        er, ec, en, key = er[order], ec[order], en[order], key[order]
        cnt = np.bincount(key, minlength=nchunk * 2).reshape(nchunk, 2)
        counts[c] = cnt
        per_core.append((er, ec, en, cnt))

    tiles = np.maximum((counts.max(axis=0) + 127) // 128, 0)
    # ensure at least one tile per chunk (half 0) so PSUM gets initialized
    zero_rows = tiles.sum(axis=1) == 0
    tiles[zero_rows, 0] = 1

    T = [int(tiles[:, h].sum()) for h in range(2)]           # tiles per half
    S = [t * 128 for t in T]                                  # slots per half

    idx_np = [np.zeros((ncores, 128, S[h] // 16), np.int16) for h in range(2)]
    nrm_np = [np.zeros((ncores, 128, T[h]), bf16) for h in range(2)]
    dst_np = [np.zeros((ncores, 128, T[h]), bf16) for h in range(2)]

    for c in range(ncores):
        er, ec, en, cnt = per_core[c]
        # slot offsets per (chunk, half) in the uniform layout
        epos = np.concatenate([[0], np.cumsum(cnt.reshape(-1))])  # within core stream
        for h in range(2):
            ivals = np.full(S[h], -1 if PAD_NEG else 0, np.int64)
            nvals = np.zeros(S[h], np.float32)
            dvals = np.zeros(S[h], np.float32)
            t_off = 0
            for ch in range(nchunk):
                k = ch * 2 + h
                cn = cnt[ch, h]
                s0 = t_off * 128
                e0 = epos[k]
                ivals[s0:s0 + cn] = er[e0:e0 + cn] - h * half
                nvals[s0:s0 + cn] = en[e0:e0 + cn]
                dvals[s0:s0 + cn] = ec[e0:e0 + cn] - ch * cfg["CHUNK"]
                t_off += tiles[ch, h]
            wrapped = ivals.astype(np.int16).reshape(-1, 16).T
            idx_np[h][c] = np.tile(wrapped, (8, 1))  # replicate per 16-part group
            nrm_np[h][c] = nvals.astype(bf16).reshape(-1, 128).T
            dst_np[h][c] = dvals.astype(bf16).reshape(-1, 128).T

    # groups of chunks
    groups = []
    g0 = 0
    while g0 < nchunk:
        g1 = min(g0 + cfg["GROUP"], nchunk)
        groups.append((g0, g1))
        g0 = g1
    return dict(tiles=tiles, groups=groups, idx=idx_np, nrm=nrm_np, dst=dst_np,
                T=T, npc=npc, nchunk=nchunk, half=half, counts=counts)


def pack_weights(inputs, cfg):
    """Host-side packing of the small ARMA weights into lhsT/rhs layouts."""
    k, hid, cls, f = cfg["K"], cfg["HID"], cfg["CLS"], cfg["F"]
    w1x, w2x = k * hid, k * cls

    def t(a):
        return np.asarray(a, np.float32)

    W0 = np.transpose(t(inputs["c1_init_w"]), (1, 0, 2)).reshape(f, w1x)
    R1 = np.transpose(t(inputs["c1_root_w"]), (1, 0, 2)).reshape(f, w1x)
    W1 = np.transpose(t(inputs["c1_w"]), (1, 0, 2)).reshape(hid, w1x)
    B1 = np.broadcast_to(t(inputs["c1_bias"]).reshape(1, w1x), (128, w1x))
    W20 = np.zeros((hid, 128), np.float32)
    W20[:, :w2x] = np.transpose(t(inputs["c2_init_w"]), (1, 0, 2)).reshape(hid, w2x)
    R2 = np.zeros((hid, 128), np.float32)
    R2[:, :w2x] = np.transpose(t(inputs["c2_root_w"]), (1, 0, 2)).reshape(hid, w2x)
    W2 = np.zeros((32, 128), np.float32)
    for kk in range(k):
        W2[kk * cls:(kk + 1) * cls, kk * cls:(kk + 1) * cls] = t(inputs["c2_w"])[kk]
    B2 = np.zeros((128, 128), np.float32)
    B2[:, :w2x] = np.broadcast_to(t(inputs["c2_bias"]).reshape(1, w2x), (128, w2x))
    return {n: v.astype(bf16) for n, v in
            dict(W0=W0, R1=R1, W1=W1, B1=B1, W20=W20, R2=R2, W2=W2, B2=B2).items()}


# ---------------------------------------------------------------- builder
def build_kernel(cfg, st, debug=False):
    import concourse.bass as bass
    import concourse.mybir as mybir
    import concourse.tile as tile
    from concourse import bacc, library_config

    n, f, hid, cls, k, g = cfg["N"], cfg["F"], cfg["HID"], cfg["CLS"], cfg["K"], cfg["G"]
    ncores = cfg["NCORES"]
    npc, nchunk = st["npc"], st["nchunk"]
    half = st["half"]
    w1x, w2x = k * hid, k * cls
    tiles, groups = st["tiles"], st["groups"]
    T = st["T"]
    dt = mybir.dt
    AOT = mybir.AluOpType
    ACT = mybir.ActivationFunctionType

    nc = bacc.Bacc(None, target_bir_lowering=False, num_devices=ncores)

    # ---------------- dram parameters
    def din(name, shape, dtype):
        return nc.dram_tensor(name, shape, dtype, kind="ExternalInput")

    xtab = din("xtab", [n, f], dt.bfloat16)
    xT = din("xT", [f, npc], dt.bfloat16)
    iota128p = din("iota128", [128, 128], dt.bfloat16)
    iota64p = din("iota64", [128, g], dt.bfloat16)
    identp = din("ident", [128, 128], dt.bfloat16)
    wts = {nm: din(nm, list(sh), dt.bfloat16) for nm, sh in dict(
        W0=(f, w1x), R1=(f, w1x), W1=(hid, w1x), B1=(128, w1x),
        W20=(hid, 128), R2=(hid, 128), W2=(32, 128), B2=(128, 128)).items()}
    idxp = [din(f"idx{h}", [128, T[h] * 8], dt.int16) for h in range(2)]
    nrmp = [din(f"nrm{h}", [128, T[h]], dt.bfloat16) for h in range(2)]
    dstp = [din(f"dst{h}", [128, T[h]], dt.bfloat16) for h in range(2)]
    gidp = din("gid", [128, nchunk], dt.bfloat16)
    call_list = enumerate_calls(st, groups)
    ncalls = len(call_list)
    gcntp = din("gcnt", [1, ncalls], dt.int32)

    out_part = nc.dram_tensor("out_part", [g, cls], dt.float32, kind="ExternalOutput")
    feat_part = nc.dram_tensor("feat_part", [g, hid], dt.float32, kind="ExternalOutput")
    if debug:
        dbg = {nm: nc.dram_tensor(nm, sh, dt.bfloat16, kind="ExternalOutput")
               for nm, sh in dict(d_out0=[n, w1x], d_h20=[n, 128], d_h21=[n, 128],
                                  d_hblk=[128, nchunk * hid],
                                  d_root1=[128, nchunk * w1x],
                                  d_root2=[128, nchunk * 128]).items()}

    rg = [list(range(ncores))]

    with tile.TileContext(nc) as tc:
        import contextlib
        with contextlib.ExitStack() as ctx:
            nc.gpsimd.load_library(library_config.mlp)
            dram = ctx.enter_context(tc.tile_pool(name="dram", bufs=1, space="DRAM"))
            cc1_in = dram.tile([npc, w1x], dt.bfloat16)
            cc1_out = dram.tile([n, w1x], dt.bfloat16, addr_space="Shared")
            cc2_in = dram.tile([npc, 128], dt.bfloat16)
            cc2_out = dram.tile([n, 128], dt.bfloat16, addr_space="Shared")
            cc3_in = dram.tile([npc, 128], dt.bfloat16)
            cc3_out = dram.tile([n, 128], dt.bfloat16, addr_space="Shared")

            const = ctx.enter_context(tc.tile_pool(name="const", bufs=1))
            iota128 = const.tile([128, 128], dt.bfloat16)
            nc.sync.dma_start(out=iota128[:], in_=iota128p[:])
            iota64 = const.tile([128, g], dt.bfloat16)
            nc.sync.dma_start(out=iota64[:], in_=iota64p[:])
            ident = const.tile([128, 128], dt.bfloat16)
            nc.sync.dma_start(out=ident[:], in_=identp[:])

            wt_sb = {}
            for nm, t_ in wts.items():
                s = const.tile(list(t_.shape), dt.bfloat16, name=f"sb_{nm}")
                nc.sync.dma_start(out=s[:], in_=t_[:])
                wt_sb[nm] = s
            xT_sb = const.tile([f, npc], dt.bfloat16)
            nc.sync.dma_start(out=xT_sb[:], in_=xT[:])
            idx_sb = []
            nrm_sb = []
            dst_sb = []
            for h in range(2):
                i_ = const.tile([128, T[h] * 8], dt.int16, name=f"idx_sb{h}")
                nc.sync.dma_start(out=i_[:], in_=idxp[h][:])
                nm_ = const.tile([128, T[h]], dt.bfloat16, name=f"nrm_sb{h}")
                nc.sync.dma_start(out=nm_[:], in_=nrmp[h][:])
                ds_ = const.tile([128, T[h]], dt.bfloat16, name=f"dst_sb{h}")
                nc.sync.dma_start(out=ds_[:], in_=dstp[h][:])
                idx_sb.append(i_)
                nrm_sb.append(nm_)
                dst_sb.append(ds_)
            gid_sb = const.tile([128, nchunk], dt.bfloat16)
            nc.sync.dma_start(out=gid_sb[:], in_=gidp[:])
            gcnt_sb = const.tile([1, ncalls], dt.int32)
            nc.sync.dma_start(out=gcnt_sb[:], in_=gcntp[:])
            cnt_reg = nc.gpsimd.alloc_register("gather_cnt")

            big = ctx.enter_context(tc.tile_pool(name="big", bufs=1))
            root1_sb = big.tile([128, nchunk * w1x], dt.bfloat16)
            root2_sb = big.tile([128, nchunk * 128], dt.bfloat16)
            hblk_sb = big.tile([128, nchunk * hid], dt.bfloat16)

            sbuf = ctx.enter_context(tc.tile_pool(name="sbuf", bufs=3))
            lhsp = ctx.enter_context(tc.tile_pool(name="lhsp", bufs=4))
            gbp = ctx.enter_context(tc.tile_pool(name="gbp", bufs=2))
            p_agg = ctx.enter_context(tc.tile_pool(name="p_agg", bufs=2, space="PSUM"))
            p_z = ctx.enter_context(tc.tile_pool(name="p_z", bufs=2, space="PSUM"))
            p_tp = ctx.enter_context(tc.tile_pool(name="p_tp", bufs=2, space="PSUM"))
            p_acc = ctx.enter_context(tc.tile_pool(name="p_acc", bufs=1, space="PSUM"))

            def cw_of(ch):
                return min(cfg["CHUNK"], npc - ch * cfg["CHUNK"])

            # ---------------- root1 = xT.T @ R1 + B1  (bf16, per chunk)
            for ch in range(nchunk):
                cw = cw_of(ch)
                rp = p_z.tile([128, w1x], dt.float32, tag="z")
                nc.tensor.matmul(out=rp[:cw, :], lhsT=xT_sb[:, ch * 128: ch * 128 + cw],
                                 rhs=wt_sb["R1"][:], start=True, stop=True)
                nc.vector.tensor_tensor(out=rp[:cw, :], in0=rp[:cw, :],
                                        in1=wt_sb["B1"][:cw, :], op=AOT.add)
                if cw < 128:
                    nc.vector.memset(root1_sb[:, ch * w1x:(ch + 1) * w1x], 0.0)
                nc.scalar.activation(out=root1_sb[:cw, ch * w1x:(ch + 1) * w1x],
                                     in_=rp[:cw, :], func=ACT.Copy)

            # ---------------- generic propagate phase
            def propagate(phase, table, elem, msg_w, epilogue):
                """table: dram AP [n, elem]; per chunk calls
                epilogue(ch, cw, agg_psum_ap [128, msg_w] fp32)."""
                # tile offset within each half stream, per chunk
                t_off = np.zeros((nchunk, 2), np.int64)
                t_off[1:] = np.cumsum(tiles[:-1], axis=0)
                ci = 0
                for gi, (g0, g1) in enumerate(groups):
                    gt = [int(tiles[g0:g1, h].sum()) for h in range(2)]
                    gb = []
                    for h in range(2):
                        nt = max(gt[h], 1)
                        b = gbp.tile([128, nt, elem], dt.bfloat16,
                                     tag=f"gb{elem}_{h}", name=f"gb{phase}_{h}_{g0}")
                        if phase in ("c1t0", "c1t1") and gi < 2:
                            # first uses of each rotating buffer slot: clear
                            # so skipped pad slots never read NaN bit patterns
                            nc.vector.memset(b[:], 0.0)
                        base = 0 if h == 0 else half
                        rows = (n - half) if h == 1 else half
                        # one (or more) calls per (chunk, half) so host padding
                        # is always trailing within a call (ucode trims it)
                        for ch in range(g0, g1):
                            ct = int(tiles[ch, h])
                            coff = int(t_off[ch, h] - t_off[g0, h])
                            for c0 in range(0, ct, MAXT):
                                c1 = min(c0 + MAXT, ct)
                                s0 = (int(t_off[ch, h]) + c0) * 128
                                nidx = (c1 - c0) * 128
                                assert call_list[ci] == (h, ch, c0, c1)
                                nc.gpsimd.reg_load(cnt_reg, gcnt_sb[0:1, ci:ci + 1])
                                nc.gpsimd.dma_gather(
                                    out_ap=b[:, coff + c0:coff + c1, :],
                                    in_ap=table[base:base + rows, :],
                                    idxs_ap=idx_sb[h][:, s0 // 16:(s0 + nidx) // 16],
                                    num_idxs=nidx,
                                    num_idxs_reg=cnt_reg,
                                    elem_size=elem,
                                )
                                ci += 1
                        gb.append(b)
                    for ch in range(g0, g1):
                        cw = cw_of(ch)
                        agg = p_agg.tile([128, w1x], dt.float32, tag="agg",
                                         name=f"agg{phase}_{ch}")
                        ntot = int(tiles[ch].sum())
                        ti = 0
                        for h in range(2):
                            for t_ in range(int(tiles[ch, h])):
                                gtile = int(t_off[ch, h] - t_off[g0, h] + t_)
                                stile = int(t_off[ch, h] + t_)
                                lhsT = lhsp.tile([128, 128], dt.bfloat16,
                                                 tag="lhsT", name=f"lh{phase}_{ch}_{h}_{t_}")
                                nc.vector.scalar_tensor_tensor(
                                    out=lhsT[:],
                                    in0=iota128[:],
                                    scalar=dst_sb[h][:, stile:stile + 1],
                                    in1=nrm_sb[h][:, stile:stile + 1].to_broadcast([128, 128]),
                                    op0=AOT.is_equal, op1=AOT.mult)
                                nc.tensor.matmul(
                                    out=agg[:, :msg_w], lhsT=lhsT[:],
                                    rhs=gb[h][:, gtile, :msg_w],
                                    start=(ti == 0), stop=(ti == ntot - 1))
                                ti += 1
                        epilogue(ch, cw, agg)

            # ---------------- conv1 t=0: gather x, agg, @W0, +root1, relu
            def epi_c1t0(ch, cw, agg):
                aggS = sbuf.tile([128, f], dt.bfloat16, tag="aggS", name=f"aggS0_{ch}")
                nc.scalar.activation(out=aggS[:], in_=agg[:, :f], func=ACT.Copy)
                aT = p_tp.tile([128, 128], dt.bfloat16, tag="tp", name=f"aT0_{ch}")
                nc.tensor.transpose(out=aT[:], in_=aggS[:], identity=ident[:])
                aTs = sbuf.tile([128, 128], dt.bfloat16, tag="aTs", name=f"aTs0_{ch}")
                nc.vector.tensor_copy(aTs[:], aT[:])
                z = p_z.tile([128, w1x], dt.float32, tag="z", name=f"z0_{ch}")
                nc.tensor.matmul(out=z[:], lhsT=aTs[:], rhs=wt_sb["W0"][:],
                                 start=True, stop=True)
                nc.vector.tensor_tensor(out=z[:cw, :], in0=z[:cw, :],
                                        in1=root1_sb[:cw, ch * w1x:(ch + 1) * w1x],
                                        op=AOT.add)
                o0 = sbuf.tile([128, w1x], dt.bfloat16, tag="obuf", name=f"o0_{ch}")
                nc.scalar.activation(out=o0[:cw, :], in_=z[:cw, :], func=ACT.Relu)
                nc.sync.dma_start(out=cc1_in[ch * 128: ch * 128 + cw, :], in_=o0[:cw, :])

            propagate("c1t0", xtab, f, f, epi_c1t0)

            nc.gpsimd.collective_compute(
                "AllGather", AOT.bypass, replica_groups=rg,
                ins=[cc1_in.opt()], outs=[cc1_out.opt()])

            # ---------------- conv1 t=1
            def epi_c1t1(ch, cw, agg):
                aggS = sbuf.tile([128, w1x], dt.bfloat16, tag="aggS", name=f"aggS1_{ch}")
                nc.scalar.activation(out=aggS[:], in_=agg[:, :w1x], func=ACT.Copy)
                z = p_z.tile([128, w1x], dt.float32, tag="z", name=f"z1_{ch}")
                for kk in range(k):
                    sl = slice(kk * hid, (kk + 1) * hid)
                    aT = p_tp.tile([128, 128], dt.bfloat16, tag="tp", name=f"aT1_{ch}_{kk}")
                    nc.tensor.transpose(out=aT[:], in_=aggS[:, sl], identity=ident[:])
                    aTs = sbuf.tile([128, 128], dt.bfloat16, tag="aTs", name=f"aTs1_{ch}_{kk}")
                    nc.vector.tensor_copy(aTs[:], aT[:])
                    nc.tensor.matmul(out=z[:, sl], lhsT=aTs[:], rhs=wt_sb["W1"][:, sl],
                                     start=True, stop=True)
                nc.vector.tensor_tensor(out=z[:cw, :], in0=z[:cw, :],
                                        in1=root1_sb[:cw, ch * w1x:(ch + 1) * w1x],
                                        op=AOT.add)
                o1 = sbuf.tile([128, w1x], dt.bfloat16, tag="obuf", name=f"o1_{ch}")
                nc.scalar.activation(out=o1[:], in_=z[:], func=ACT.Relu)
                # h = mean over stacks (padded rows of z are zero+root(=0) -> relu 0)
                tmp = sbuf.tile([128, hid], dt.float32, tag="tmp", name=f"tm1_{ch}")
                nc.vector.tensor_tensor(out=tmp[:], in0=o1[:, 0:hid],
                                        in1=o1[:, hid:2 * hid], op=AOT.add)
                nc.vector.tensor_tensor(out=tmp[:], in0=tmp[:],
                                        in1=o1[:, 2 * hid:3 * hid], op=AOT.add)
                hsl = hblk_sb[:, ch * hid:(ch + 1) * hid]
                nc.vector.tensor_scalar_mul(hsl, tmp[:], 1.0 / 3.0)
                # conv2 prework on this chunk
                hT = p_tp.tile([128, 128], dt.bfloat16, tag="tp", name=f"hT_{ch}")
                nc.tensor.transpose(out=hT[:], in_=hsl, identity=ident[:])
                hTs = sbuf.tile([128, 128], dt.bfloat16, tag="aTs", name=f"hTs_{ch}")
                nc.vector.tensor_copy(hTs[:], hT[:])
                h20 = p_z.tile([128, w1x], dt.float32, tag="z", name=f"h20_{ch}")
                nc.tensor.matmul(out=h20[:, :128], lhsT=hTs[:], rhs=wt_sb["W20"][:],
                                 start=True, stop=True)
                o20 = sbuf.tile([128, 128], dt.bfloat16, tag="o2buf", name=f"o20_{ch}")
                nc.scalar.activation(out=o20[:], in_=h20[:, :128], func=ACT.Copy)
                nc.sync.dma_start(out=cc2_in[ch * 128: ch * 128 + cw, :], in_=o20[:cw, :])
                r2 = p_z.tile([128, w1x], dt.float32, tag="z", name=f"r2_{ch}")
                nc.tensor.matmul(out=r2[:cw, :128], lhsT=hTs[:, :cw],
                                 rhs=wt_sb["R2"][:], start=True, stop=True)
                nc.vector.tensor_tensor(out=r2[:cw, :128], in0=r2[:cw, :128],
                                        in1=wt_sb["B2"][:cw, :], op=AOT.add)
                if cw < 128:
                    nc.vector.memset(root2_sb[:, ch * 128:(ch + 1) * 128], 0.0)
                nc.scalar.activation(out=root2_sb[:cw, ch * 128:(ch + 1) * 128],
                                     in_=r2[:cw, :128], func=ACT.Copy)
                # features pool
                oh = lhsp.tile([128, g], dt.bfloat16, tag="oh", name=f"oh_{ch}")
                nc.vector.tensor_tensor(out=oh[:], in0=gid_sb[:, ch:ch + 1].to_broadcast([128, g]),
                                        in1=iota64[:], op=AOT.is_equal)
                nc.tensor.matmul(out=featp[:, :], lhsT=oh[:], rhs=hsl,
                                 start=(ch == 0), stop=(ch == nchunk - 1),
                                 skip_group_check=True)

            featp = p_acc.tile([g, hid], dt.float32, tag="featp")
            propagate("c1t1", cc1_out, w1x, w1x, epi_c1t1)

            nc.gpsimd.collective_compute(
                "AllGather", AOT.bypass, replica_groups=rg,
                ins=[cc2_in.opt()], outs=[cc2_out.opt()])

            # ---------------- conv2 t=0: gather h2_0 table, z=agg+root2 -> h2_1
            def epi_c2t0(ch, cw, agg):
                z = sbuf.tile([128, w2x], dt.bfloat16, tag="tmp2", name=f"z20_{ch}")
                nc.vector.tensor_tensor(out=z[:], in0=agg[:, :w2x],
                                        in1=root2_sb[:, ch * 128: ch * 128 + w2x],
                                        op=AOT.add)
                oT = p_tp.tile([128, 128], dt.bfloat16, tag="tp", name=f"oT2_{ch}")
                nc.tensor.transpose(out=oT[:w2x, :], in_=z[:], identity=ident[:])
                oTs = sbuf.tile([32, 128], dt.bfloat16, tag="oTs", name=f"oTs2_{ch}")
                nc.vector.tensor_copy(oTs[:w2x, :], oT[:w2x, :])
                h21 = p_z.tile([128, w1x], dt.float32, tag="z", name=f"h21_{ch}")
                nc.tensor.matmul(out=h21[:, :128], lhsT=oTs[:w2x, :],
                                 rhs=wt_sb["W2"][:w2x, :], start=True, stop=True)
                o21 = sbuf.tile([128, 128], dt.bfloat16, tag="o2buf", name=f"o21_{ch}")
                nc.scalar.activation(out=o21[:], in_=h21[:, :128], func=ACT.Copy)
                nc.sync.dma_start(out=cc3_in[ch * 128: ch * 128 + cw, :], in_=o21[:cw, :])

            propagate("c2t0", cc2_out, 128, w2x, epi_c2t0)

            nc.gpsimd.collective_compute(
                "AllGather", AOT.bypass, replica_groups=rg,
                ins=[cc3_in.opt()], outs=[cc3_out.opt()])

            # ---------------- conv2 t=1: z=agg+root2, mean stacks, pool
            def epi_c2t1(ch, cw, agg):
                z2 = sbuf.tile([128, w2x], dt.float32, tag="z2t1", name=f"z2t1_{ch}")
                nc.vector.tensor_tensor(out=z2[:], in0=agg[:, :w2x],
                                        in1=root2_sb[:, ch * 128: ch * 128 + w2x],
                                        op=AOT.add)
                tmp = sbuf.tile([128, cls], dt.float32, tag="tmp2", name=f"tm2_{ch}")
                nc.vector.tensor_tensor(out=tmp[:], in0=z2[:, 0:cls],
                                        in1=z2[:, cls:2 * cls], op=AOT.add)
                nc.vector.tensor_tensor(out=tmp[:], in0=tmp[:],
                                        in1=z2[:, 2 * cls:3 * cls], op=AOT.add)
                zm = sbuf.tile([128, cls], dt.bfloat16, tag="zm", name=f"zm_{ch}")
                nc.vector.tensor_scalar_mul(zm[:], tmp[:], 1.0 / 3.0)
                oh = lhsp.tile([128, g], dt.bfloat16, tag="oh", name=f"oh2_{ch}")
                nc.vector.tensor_tensor(out=oh[:], in0=gid_sb[:, ch:ch + 1].to_broadcast([128, g]),
                                        in1=iota64[:], op=AOT.is_equal)
                nc.tensor.matmul(out=outp[:, :], lhsT=oh[:], rhs=zm[:],
                                 start=(ch == 0), stop=(ch == nchunk - 1),
                                 skip_group_check=True)

            outp = p_acc.tile([g, cls], dt.float32, tag="outp")
            propagate("c2t1", cc3_out, 128, w2x, epi_c2t1)

            if debug:
                nc.sync.dma_start(out=dbg["d_out0"][:], in_=cc1_out[:])
                nc.sync.dma_start(out=dbg["d_h20"][:], in_=cc2_out[:])
                nc.sync.dma_start(out=dbg["d_h21"][:], in_=cc3_out[:])
                nc.sync.dma_start(out=dbg["d_hblk"][:], in_=hblk_sb[:])
                nc.sync.dma_start(out=dbg["d_root1"][:], in_=root1_sb[:])
                nc.sync.dma_start(out=dbg["d_root2"][:], in_=root2_sb[:])

            # ---------------- finals
            featS = sbuf.tile([g, hid], dt.float32, tag="featS")
            nc.vector.tensor_copy(featS[:], featp[:])
            nc.sync.dma_start(out=feat_part[:], in_=featS[:])
            outS = sbuf.tile([g, cls], dt.float32, tag="outS")
            nc.vector.tensor_copy(outS[:], outp[:])
            nc.sync.dma_start(out=out_part[:], in_=outS[:])

    nc.compile()
    return nc


# ---------------------------------------------------------------- in_maps
def build_in_maps(inputs, cfg, st):
    n, f = cfg["N"], cfg["F"]
    ncores = cfg["NCORES"]
    npc = st["npc"]
    nchunk = st["nchunk"]
    x = np.asarray(inputs["x"], np.float32)
    batch = np.asarray(inputs["batch"], np.int64)
    wt = pack_weights(inputs, cfg)
    x_bf = x.astype(bf16)

    in_maps = []
    ar = np.arange(128, dtype=np.float32)
    iota128 = np.broadcast_to(ar[None, :], (128, 128)).astype(bf16)
    iota64 = np.broadcast_to(ar[None, :cfg["G"]], (128, cfg["G"])).astype(bf16)
    ident = np.eye(128, dtype=np.float32).astype(bf16)
    for c in range(ncores):
        lo = c * npc
        m = dict(xtab=x_bf,
                 xT=np.ascontiguousarray(x_bf[lo:lo + npc].T),
                 iota128=iota128, iota64=iota64, ident=ident)
        m.update(wt)
        for h in range(2):
            m[f"idx{h}"] = np.ascontiguousarray(st["idx"][h][c])
            m[f"nrm{h}"] = np.ascontiguousarray(st["nrm"][h][c])
            m[f"dst{h}"] = np.ascontiguousarray(st["dst"][h][c])
        gid = np.zeros((128, nchunk), bf16)
        gv = batch[lo:lo + npc].astype(np.float32)
        gv = np.concatenate([gv, np.zeros(nchunk * 128 - npc, np.float32)])
        gid[:, :] = gv.reshape(nchunk, 128).T.astype(bf16)
        m["gid"] = gid
        calls = enumerate_calls(st, st["groups"])
        gcnt = np.zeros((1, len(calls)), np.int32)
        for ci, (h, ch, c0, c1) in enumerate(calls):
            nidx = (c1 - c0) * 128
            if PAD_NEG:
                true = int(st["counts"][c, ch, h])
                gcnt[0, ci] = min(max(true - c0 * 128, 0), nidx)
            else:
                gcnt[0, ci] = nidx
        m["gcnt"] = gcnt
        in_maps.append(m)
    return in_maps


_CACHE = {}


def kernel(**inputs):
    from concourse.bass_utils import run_bass_kernel_spmd

    cfg = DEF_CFG
    edge_index = np.asarray(inputs["edge_index"])
    st = host_prep(edge_index, cfg)
    nc = build_kernel(cfg, st)
    in_maps = build_in_maps(inputs, cfg, st)
    res = run_bass_kernel_spmd(nc, in_maps, core_ids=list(range(cfg["NCORES"])))
    out = np.zeros((cfg["G"], cfg["CLS"]), np.float32)
    feat = np.zeros((cfg["G"], cfg["HID"]), np.float32)
    for r in res.results:
        out += np.asarray(r["out_part"], np.float32)
        feat += np.asarray(r["feat_part"], np.float32)
    return out, feat
